# revision 6
# baseline (speedup 1.0000x reference)
"""Trainium2 Bass kernel for the per-head channel-attention module.

Math (per batch b, all in fp32):
  Q = emb @ Wq[h].T, K = emb @ Wk[h].T        [N, C] each
  scores_h = Q.T @ K / sqrt(C)                [C, C]
  normed = InstanceNorm2d(scores)             (per (b,h) map, biased var)
  probs = softmax(normed, axis=-1)
  weights = mean_h probs                      [C, C]   (output 2)
  ctx = (1/H) sum_h probs_h @ V_h.T-ish ; O1 = ctx @ Wout.T   [N, C] (output 1)

Key restructure: scores contract over tokens N, so
  scores_h = Wq_h @ G' @ Wk_h.T   with G' = (emb.T @ emb) / sqrt(C)  [C, C]
and the output path collapses to
  O1 = emb @ Z,  Z = (1/H * sum_h probs_h @ Wv_h).T-chained with Wout:
  S'[i,c] = sum_h (probs_h @ Wv_h)[i,c];  Z[c,d] = sum_i S'[i,c]/H * Wout[d,i]
This turns ~11.3 GFLOP/batch into ~1.5 GFLOP/batch.

Sharding: data-parallel, one batch per NeuronCore (B=8, 8 cores).
Host side pre-transposes the small weight matrices and emb (embT) so every
matmul operand lands with its contraction dim on SBUF partitions.
Per-core outputs: o1T = O1[b].T (host transposes back) and wts = weights[b].
"""

import os

import numpy as np

import concourse.bacc as bacc
import concourse.bass as bass
import concourse.mybir as mybir
import concourse.tile as tile
from concourse.bass_utils import run_bass_kernel_spmd
from concourse.masks import make_identity

B, N, C, H = 8, 4096, 256, 4
EPS = 1e-5
P = 128
TC = C // P          # 2 c-tiles
KT = N // P          # 32 token-tiles
NCH = N // 512       # 8 chunks of 512 tokens for the final matmul
F32 = mybir.dt.float32

# Matmul dtype knobs. float32 = exact (4 cyc/row); float32r = fast (1 cyc/row
# at free dim >= 256) with relaxed multiply precision. Overridable via env.
_DT_MAP = {"float32": mybir.dt.float32, "float32r": mybir.dt.float32r}
MM_BIG = _DT_MAP[os.environ.get("ATT_MM_BIG", "float32")]      # Gram + O1 matmuls
MM_SMALL = _DT_MAP[os.environ.get("ATT_MM_SMALL", "float32")]  # 256^3 head matmuls


def _mm(ap, dt):
    return ap.bitcast(dt) if dt != F32 else ap


def build_bass():
    nc = bacc.Bacc(None, target_bir_lowering=False)

    emb_h = nc.dram_tensor("emb", [N, C], F32, kind="ExternalInput")
    embT_h = nc.dram_tensor("embT", [C, N], F32, kind="ExternalInput")
    wqT_h = nc.dram_tensor("wqT", [H, C, C], F32, kind="ExternalInput")
    wkT_h = nc.dram_tensor("wkT", [H, C, C], F32, kind="ExternalInput")
    wv_h = nc.dram_tensor("wv", [H, C, C], F32, kind="ExternalInput")
    woutT_h = nc.dram_tensor("woutT", [C, C], F32, kind="ExternalInput")
    o1T_h = nc.dram_tensor("o1T", [C, N], F32, kind="ExternalOutput")
    wts_h = nc.dram_tensor("wts", [C, C], F32, kind="ExternalOutput")

    with tile.TileContext(nc) as tc:
        with (
            tc.tile_pool(name="singles", bufs=1) as singles,
            tc.tile_pool(name="perhead", bufs=2) as perhead,
            tc.tile_pool(name="outs", bufs=3) as outs,
            tc.tile_pool(name="ps", bufs=6, space="PSUM") as ps,
            tc.tile_pool(name="acc", bufs=2, space="PSUM") as acc,
        ):
            # ---- resident SBUF tensors -------------------------------------
            emb_sb = singles.tile([P, KT, C], F32)       # emb[t*128+p, c]
            embT_sb = singles.tile([P, TC, N], F32)      # emb[n, t*128+p]
            wqT_sb = singles.tile([P, H, TC, C], F32)    # Wq[h, d, t*128+p]
            wkT_sb = singles.tile([P, H, TC, C], F32)
            wv_sb = singles.tile([P, H, TC, C], F32)     # Wv[h, t*128+p, c]
            woutT_sb = singles.tile([P, TC, C], F32)     # Wout[d, t*128+p]
            G_sb = singles.tile([P, TC, C], F32)         # G'/sqrt(C)
            S_sb = singles.tile([P, TC, C], F32)         # S'/H
            Z_sb = singles.tile([P, TC, C], F32)
            wacc_sb = singles.tile([P, TC, C], F32)      # sum_h probs
            ones_sb = singles.tile([P, P], F32)
            ident_sb = singles.tile([P, P], F32)
            eps_sb = singles.tile([P, 1], F32)

            nc.vector.memset(ones_sb[:], 1.0)
            nc.vector.memset(eps_sb[:], EPS)
            nc.vector.memset(wacc_sb[:], 0.0)
            make_identity(nc, ident_sb[:])

            # ---- input DMAs -------------------------------------------------
            nc.sync.dma_start(
                out=emb_sb[:], in_=emb_h[:].rearrange("(t p) c -> p t c", p=P)
            )
            nc.sync.dma_start(
                out=embT_sb[:], in_=embT_h[:].rearrange("(t p) n -> p t n", p=P)
            )
            nc.sync.dma_start(
                out=wqT_sb[:], in_=wqT_h[:].rearrange("h (t p) d -> p h t d", p=P)
            )
            nc.sync.dma_start(
                out=wkT_sb[:], in_=wkT_h[:].rearrange("h (t p) d -> p h t d", p=P)
            )
            nc.sync.dma_start(
                out=wv_sb[:], in_=wv_h[:].rearrange("h (t p) c -> p h t c", p=P)
            )
            nc.sync.dma_start(
                out=woutT_sb[:], in_=woutT_h[:].rearrange("(t p) d -> p t d", p=P)
            )

            # ---- Gram: G = emb.T @ emb, scaled by 1/sqrt(C) ----------------
            for mi in range(TC):
                g_ps = ps.tile([P, C], F32, tag="ps")
                for k in range(KT):
                    nc.tensor.matmul(
                        g_ps[:],
                        lhsT=_mm(emb_sb[:, k, mi * P : (mi + 1) * P], MM_BIG),
                        rhs=_mm(emb_sb[:, k, :], MM_BIG),
                        start=(k == 0),
                        stop=(k == KT - 1),
                    )
                nc.scalar.mul(G_sb[:, mi, :], g_ps[:], 1.0 / 16.0)

            # S' accumulator lives across the whole head loop
            s_acc = [
                acc.tile([P, C], F32, tag="acc", name=f"sacc{i}") for i in range(TC)
            ]

            inv_cc = 1.0 / float(C * C)

            for h in range(H):
                # U = G' @ WkT_h   [c, j], c on partitions
                U_sb = perhead.tile([P, TC, C], F32, tag="u")
                for mc in range(TC):
                    u_ps = ps.tile([P, C], F32, tag="ps")
                    for kc in range(TC):
                        nc.tensor.matmul(
                            u_ps[:],
                            lhsT=_mm(G_sb[:, kc, mc * P : (mc + 1) * P], MM_SMALL),
                            rhs=_mm(wkT_sb[:, h, kc, :], MM_SMALL),
                            start=(kc == 0),
                            stop=(kc == TC - 1),
                        )
                    nc.vector.tensor_copy(out=U_sb[:, mc, :], in_=u_ps[:])

                # scores = Wq_h @ U   [i, j], i on partitions (stays in PSUM)
                sc_ps = []
                for mi in range(TC):
                    p_ = ps.tile([P, C], F32, tag="ps")
                    for kc in range(TC):
                        nc.tensor.matmul(
                            p_[:],
                            lhsT=_mm(wqT_sb[:, h, kc, mi * P : (mi + 1) * P], MM_SMALL),
                            rhs=_mm(U_sb[:, kc, :], MM_SMALL),
                            start=(kc == 0),
                            stop=(kc == TC - 1),
                        )
                    sc_ps.append(p_)

                # instance-norm stats: need r = 1/sqrt(var + eps) only
                stat_sb = perhead.tile([P, 4], F32, tag="stat")
                rmax_sb = perhead.tile([P, TC], F32, tag="rmax")
                sq_scratch = perhead.tile([P, C], F32, tag="sqs")
                for mi in range(TC):
                    nc.vector.reduce_sum(
                        out=stat_sb[:, mi : mi + 1],
                        in_=sc_ps[mi][:],
                        axis=mybir.AxisListType.X,
                    )
                    nc.scalar.activation(
                        out=sq_scratch[:],
                        in_=sc_ps[mi][:],
                        func=mybir.ActivationFunctionType.Square,
                        accum_out=stat_sb[:, 2 + mi : 3 + mi],
                    )
                    nc.vector.reduce_max(
                        out=rmax_sb[:, mi : mi + 1],
                        in_=sc_ps[mi][:],
                        axis=mybir.AxisListType.X,
                    )

                # column-sum via ones-matmul; lhsT=[128,128] ones replicates the
                # result on every partition (free broadcast)
                cs_ps = ps.tile([P, 4], F32, tag="ps")
                nc.tensor.matmul(
                    cs_ps[:], lhsT=ones_sb[:], rhs=stat_sb[:], start=True, stop=True
                )

                cs_sb = perhead.tile([P, 4], F32, tag="cs_sb")
                nc.vector.tensor_copy(out=cs_sb[:], in_=cs_ps[:])
                scal = perhead.tile([P, 8], F32, tag="scal")
                # scal: 0=mu, 1=Esq, 2=mu^2, 3=var, 4=sd, 5=r, 6=negr
                nc.vector.tensor_tensor(
                    out=scal[:, 0:1], in0=cs_sb[:, 0:1], in1=cs_sb[:, 1:2],
                    op=mybir.AluOpType.add,
                )
                nc.scalar.mul(scal[:, 0:1], scal[:, 0:1], inv_cc)
                nc.vector.tensor_tensor(
                    out=scal[:, 1:2], in0=cs_sb[:, 2:3], in1=cs_sb[:, 3:4],
                    op=mybir.AluOpType.add,
                )
                nc.scalar.mul(scal[:, 1:2], scal[:, 1:2], inv_cc)
                nc.vector.tensor_mul(out=scal[:, 2:3], in0=scal[:, 0:1], in1=scal[:, 0:1])
                nc.vector.tensor_tensor(
                    out=scal[:, 3:4], in0=scal[:, 1:2], in1=scal[:, 2:3],
                    op=mybir.AluOpType.subtract,
                )
                nc.scalar.activation(
                    out=scal[:, 4:5], in_=scal[:, 3:4],
                    func=mybir.ActivationFunctionType.Sqrt,
                    bias=eps_sb[:],
                )
                nc.vector.reciprocal(out=scal[:, 5:6], in_=scal[:, 4:5])
                nc.scalar.mul(scal[:, 6:7], scal[:, 5:6], -1.0)

                # bias_i = -r * rowmax_i ; probs = exp(r*s + bias) (row-sum fused)
                nb_sb = perhead.tile([P, TC], F32, tag="nb")
                se_sb = perhead.tile([P, 2 * TC], F32, tag="se")
                probs_sb = perhead.tile([P, TC, C], F32, tag="probs")
                for mi in range(TC):
                    nc.vector.tensor_mul(
                        out=nb_sb[:, mi : mi + 1],
                        in0=rmax_sb[:, mi : mi + 1],
                        in1=scal[:, 6:7],
                    )
                    nc.scalar.activation(
                        out=probs_sb[:, mi, :],
                        in_=sc_ps[mi][:],
                        func=mybir.ActivationFunctionType.Exp,
                        bias=nb_sb[:, mi : mi + 1],
                        scale=scal[:, 5:6],
                        accum_out=se_sb[:, mi : mi + 1],
                    )
                nc.vector.reciprocal(
                    out=se_sb[:, TC : 2 * TC], in_=se_sb[:, 0:TC]
                )
                for mi in range(TC):
                    nc.vector.tensor_scalar_mul(
                        probs_sb[:, mi, :],
                        probs_sb[:, mi, :],
                        se_sb[:, TC + mi : TC + mi + 1],
                    )
                    # accumulate the head-mean attention map on gpsimd (idle)
                    nc.gpsimd.tensor_add(
                        out=wacc_sb[:, mi, :],
                        in0=wacc_sb[:, mi, :],
                        in1=probs_sb[:, mi, :],
                    )

                # probsT via PE transpose (fp32), then S' += probsT.T-chain @ Wv_h
                probsT_sb = perhead.tile([P, TC, C], F32, tag="probsT")
                for ti in range(TC):
                    for tj in range(TC):
                        t_ps = ps.tile([P, P], F32, tag="ps")
                        nc.tensor.transpose(
                            t_ps[:],
                            probs_sb[:, ti, tj * P : (tj + 1) * P],
                            ident_sb[:],
                        )
                        nc.vector.tensor_copy(
                            out=probsT_sb[:, tj, ti * P : (ti + 1) * P], in_=t_ps[:]
                        )

                for mi in range(TC):
                    for kj in range(TC):
                        nc.tensor.matmul(
                            s_acc[mi][:],
                            lhsT=_mm(probsT_sb[:, kj, mi * P : (mi + 1) * P], MM_SMALL),
                            rhs=_mm(wv_sb[:, h, kj, :], MM_SMALL),
                            start=(h == 0 and kj == 0),
                            stop=(h == H - 1 and kj == TC - 1),
                        )

            # ---- epilogue ---------------------------------------------------
            for mi in range(TC):
                nc.scalar.mul(S_sb[:, mi, :], s_acc[mi][:], 1.0 / H)
                nc.scalar.mul(wacc_sb[:, mi, :], wacc_sb[:, mi, :], 1.0 / H)
            nc.sync.dma_start(
                out=wts_h[:].rearrange("(t p) j -> p t j", p=P), in_=wacc_sb[:]
            )

            # Z[c,d] = sum_i S[i,c] * Wout[d,i]
            for mc in range(TC):
                z_ps = ps.tile([P, C], F32, tag="ps")
                for ki in range(TC):
                    nc.tensor.matmul(
                        z_ps[:],
                        lhsT=_mm(S_sb[:, ki, mc * P : (mc + 1) * P], MM_SMALL),
                        rhs=_mm(woutT_sb[:, ki, :], MM_SMALL),
                        start=(ki == 0),
                        stop=(ki == TC - 1),
                    )
                nc.scalar.mul(Z_sb[:, mc, :], z_ps[:], 1.0)

            # O1.T[d, n] = sum_c Z[c, d] * embT[c, n]
            for md in range(TC):
                for nch in range(NCH):
                    o_ps = ps.tile([P, 512], F32, tag="ps")
                    for kc in range(TC):
                        nc.tensor.matmul(
                            o_ps[:],
                            lhsT=_mm(Z_sb[:, kc, md * P : (md + 1) * P], MM_BIG),
                            rhs=_mm(embT_sb[:, kc, nch * 512 : (nch + 1) * 512], MM_BIG),
                            start=(kc == 0),
                            stop=(kc == TC - 1),
                        )
                    o_sb = outs.tile([P, 512], F32, tag="o1")
                    nc.vector.tensor_copy(out=o_sb[:], in_=o_ps[:])
                    nc.sync.dma_start(
                        out=o1T_h[:][
                            md * P : (md + 1) * P, nch * 512 : (nch + 1) * 512
                        ],
                        in_=o_sb[:],
                    )

    nc.compile()
    return nc


_NC_CACHE = None


def kernel(emb1, Wq, Wk, Wv, Wout):
    global _NC_CACHE
    emb1 = np.ascontiguousarray(np.asarray(emb1, dtype=np.float32))
    Wq = np.asarray(Wq, dtype=np.float32)
    Wk = np.asarray(Wk, dtype=np.float32)
    Wv = np.asarray(Wv, dtype=np.float32)
    Wout = np.asarray(Wout, dtype=np.float32)

    wqT = np.ascontiguousarray(Wq.transpose(0, 2, 1))
    wkT = np.ascontiguousarray(Wk.transpose(0, 2, 1))
    wv = np.ascontiguousarray(Wv)
    woutT = np.ascontiguousarray(Wout.T)

    if _NC_CACHE is None:
        _NC_CACHE = build_bass()
    nc = _NC_CACHE

    in_maps = []
    for b in range(B):
        in_maps.append(
            {
                "emb": emb1[b],
                "embT": np.ascontiguousarray(emb1[b].T),
                "wqT": wqT,
                "wkT": wkT,
                "wv": wv,
                "woutT": woutT,
            }
        )

    res = run_bass_kernel_spmd(nc, in_maps, core_ids=list(range(B)))

    O1 = np.empty((B, N, C), dtype=np.float32)
    weights = np.empty((B, C, C), dtype=np.float32)
    for b in range(B):
        O1[b] = res.results[b]["o1T"].T
        weights[b] = res.results[b]["wts"]
    return O1, weights


# revision 8
# speedup vs baseline: 1.3253x; 1.3253x over previous
"""Trainium2 Bass kernel for the per-head channel-attention module.

Math (per batch b, all in fp32):
  Q = emb @ Wq[h].T, K = emb @ Wk[h].T        [N, C] each
  scores_h = Q.T @ K / sqrt(C)                [C, C]
  normed = InstanceNorm2d(scores)             (per (b,h) map, biased var)
  probs = softmax(normed, axis=-1)
  weights = mean_h probs                      [C, C]   (output 2)
  ctx = (1/H) sum_h probs_h @ V_h.T-ish ; O1 = ctx @ Wout.T   [N, C] (output 1)

Key restructure: scores contract over tokens N, so
  scores_h = Wq_h @ G' @ Wk_h.T   with G' = (emb.T @ emb) / sqrt(C)  [C, C]
and the output path collapses to
  O1 = emb @ Z,  Z = (1/H * sum_h probs_h @ Wv_h).T-chained with Wout:
  S'[i,c] = sum_h (probs_h @ Wv_h)[i,c];  Z[c,d] = sum_i S'[i,c]/H * Wout[d,i]
This turns ~11.3 GFLOP/batch into ~1.5 GFLOP/batch.

Sharding: data-parallel, one batch per NeuronCore (B=8, 8 cores).
Host side pre-transposes the small weight matrices and emb (embT) so every
matmul operand lands with its contraction dim on SBUF partitions.
Per-core outputs: o1T = O1[b].T (host transposes back) and wts = weights[b].
"""

import os

import numpy as np

import concourse.bacc as bacc
import concourse.bass as bass
import concourse.mybir as mybir
import concourse.tile as tile
from concourse.bass_utils import run_bass_kernel_spmd
from concourse.masks import make_identity

B, N, C, H = 8, 4096, 256, 4
EPS = 1e-5
P = 128
TC = C // P          # 2 c-tiles
KT = N // P          # 32 token-tiles
NCH = N // 512       # 8 chunks of 512 tokens for the final matmul
F32 = mybir.dt.float32

# Matmul dtype knobs. float32 = exact (4 cyc/row); float32r = fast (1 cyc/row
# at free dim >= 256) with relaxed multiply precision. Overridable via env.
_DT_MAP = {"float32": mybir.dt.float32, "float32r": mybir.dt.float32r}
MM_BIG = _DT_MAP[os.environ.get("ATT_MM_BIG", "float32")]      # Gram + O1 matmuls
MM_SMALL = _DT_MAP[os.environ.get("ATT_MM_SMALL", "float32")]  # 256^3 head matmuls


DT_BIG = MM_BIG      # dtype for emb/embT tiles (Gram + O1 matmuls)
DT_SMALL = MM_SMALL  # dtype for weight/G/U/probsT/S/Z tiles (256^3 matmuls)


def _mm(ap, dt):
    # tiles already carry the matmul dtype; kept for call-site compatibility
    return ap


def build_bass():
    nc = bacc.Bacc(None, target_bir_lowering=False)

    emb_h = nc.dram_tensor("emb", [N, C], DT_BIG, kind="ExternalInput")
    embT_h = nc.dram_tensor("embT", [C, N], DT_BIG, kind="ExternalInput")
    wqT_h = nc.dram_tensor("wqT", [H, C, C], DT_SMALL, kind="ExternalInput")
    wkT_h = nc.dram_tensor("wkT", [H, C, C], DT_SMALL, kind="ExternalInput")
    wv_h = nc.dram_tensor("wv", [H, C, C], DT_SMALL, kind="ExternalInput")
    woutT_h = nc.dram_tensor("woutT", [C, C], DT_SMALL, kind="ExternalInput")
    o1T_h = nc.dram_tensor("o1T", [C, N], F32, kind="ExternalOutput")
    wts_h = nc.dram_tensor("wts", [C, C], F32, kind="ExternalOutput")

    with tile.TileContext(nc) as tc:
        with (
            tc.tile_pool(name="singles", bufs=1) as singles,
            tc.tile_pool(name="perhead", bufs=2) as perhead,
            tc.tile_pool(name="outs", bufs=3) as outs,
            tc.tile_pool(name="ps", bufs=6, space="PSUM") as ps,
            tc.tile_pool(name="acc", bufs=2, space="PSUM") as acc,
        ):
            # ---- resident SBUF tensors -------------------------------------
            emb_sb = singles.tile([P, KT, C], DT_BIG)       # emb[t*128+p, c]
            embT_sb = singles.tile([P, TC, N], DT_BIG)      # emb[n, t*128+p]
            wqT_sb = singles.tile([P, H, TC, C], DT_SMALL)    # Wq[h, d, t*128+p]
            wkT_sb = singles.tile([P, H, TC, C], DT_SMALL)
            wv_sb = singles.tile([P, H, TC, C], DT_SMALL)     # Wv[h, t*128+p, c]
            woutT_sb = singles.tile([P, TC, C], DT_SMALL)     # Wout[d, t*128+p]
            G_sb = singles.tile([P, TC, C], DT_SMALL)         # G'/sqrt(C)
            S_sb = singles.tile([P, TC, C], DT_SMALL)         # S'/H
            Z_sb = singles.tile([P, TC, C], DT_BIG)
            wacc_sb = singles.tile([P, TC, C], F32)      # sum_h probs
            ones_sb = singles.tile([P, P], F32)
            ident_sb = singles.tile([P, P], F32)
            eps_sb = singles.tile([P, 1], F32)

            nc.vector.memset(ones_sb[:], 1.0)
            nc.vector.memset(eps_sb[:], EPS)
            nc.vector.memset(wacc_sb[:], 0.0)
            make_identity(nc, ident_sb[:])

            # ---- input DMAs -------------------------------------------------
            nc.sync.dma_start(
                out=emb_sb[:], in_=emb_h[:].rearrange("(t p) c -> p t c", p=P)
            )
            nc.sync.dma_start(
                out=embT_sb[:], in_=embT_h[:].rearrange("(t p) n -> p t n", p=P)
            )
            nc.sync.dma_start(
                out=wqT_sb[:], in_=wqT_h[:].rearrange("h (t p) d -> p h t d", p=P)
            )
            nc.sync.dma_start(
                out=wkT_sb[:], in_=wkT_h[:].rearrange("h (t p) d -> p h t d", p=P)
            )
            nc.sync.dma_start(
                out=wv_sb[:], in_=wv_h[:].rearrange("h (t p) c -> p h t c", p=P)
            )
            nc.sync.dma_start(
                out=woutT_sb[:], in_=woutT_h[:].rearrange("(t p) d -> p t d", p=P)
            )

            # ---- Gram: G = emb.T @ emb, scaled by 1/sqrt(C) ----------------
            for mi in range(TC):
                g_ps = ps.tile([P, C], F32, tag="ps")
                for k in range(KT):
                    nc.tensor.matmul(
                        g_ps[:],
                        lhsT=_mm(emb_sb[:, k, mi * P : (mi + 1) * P], MM_BIG),
                        rhs=_mm(emb_sb[:, k, :], MM_BIG),
                        start=(k == 0),
                        stop=(k == KT - 1),
                    )
                nc.scalar.mul(G_sb[:, mi, :], g_ps[:], 1.0 / 16.0)

            # S' accumulator lives across the whole head loop
            s_acc = [
                acc.tile([P, C], F32, tag="acc", name=f"sacc{i}") for i in range(TC)
            ]

            inv_cc = 1.0 / float(C * C)

            for h in range(H):
                # U = G' @ WkT_h   [c, j], c on partitions
                U_sb = perhead.tile([P, TC, C], DT_SMALL, tag="u")
                for mc in range(TC):
                    u_ps = ps.tile([P, C], F32, tag="ps")
                    for kc in range(TC):
                        nc.tensor.matmul(
                            u_ps[:],
                            lhsT=_mm(G_sb[:, kc, mc * P : (mc + 1) * P], MM_SMALL),
                            rhs=_mm(wkT_sb[:, h, kc, :], MM_SMALL),
                            start=(kc == 0),
                            stop=(kc == TC - 1),
                        )
                    nc.vector.tensor_copy(out=U_sb[:, mc, :], in_=u_ps[:])

                # scores = Wq_h @ U   [i, j], i on partitions (stays in PSUM)
                sc_ps = []
                for mi in range(TC):
                    p_ = ps.tile([P, C], F32, tag="ps")
                    for kc in range(TC):
                        nc.tensor.matmul(
                            p_[:],
                            lhsT=_mm(wqT_sb[:, h, kc, mi * P : (mi + 1) * P], MM_SMALL),
                            rhs=_mm(U_sb[:, kc, :], MM_SMALL),
                            start=(kc == 0),
                            stop=(kc == TC - 1),
                        )
                    sc_ps.append(p_)

                # instance-norm stats: need r = 1/sqrt(var + eps) only
                stat_sb = perhead.tile([P, 4], F32, tag="stat")
                rmax_sb = perhead.tile([P, TC], F32, tag="rmax")
                sq_scratch = perhead.tile([P, C], F32, tag="sqs")
                for mi in range(TC):
                    nc.vector.reduce_sum(
                        out=stat_sb[:, mi : mi + 1],
                        in_=sc_ps[mi][:],
                        axis=mybir.AxisListType.X,
                    )
                    nc.scalar.activation(
                        out=sq_scratch[:],
                        in_=sc_ps[mi][:],
                        func=mybir.ActivationFunctionType.Square,
                        accum_out=stat_sb[:, 2 + mi : 3 + mi],
                    )
                    nc.vector.reduce_max(
                        out=rmax_sb[:, mi : mi + 1],
                        in_=sc_ps[mi][:],
                        axis=mybir.AxisListType.X,
                    )

                # column-sum via ones-matmul; lhsT=[128,128] ones replicates the
                # result on every partition (free broadcast)
                cs_ps = ps.tile([P, 4], F32, tag="ps")
                nc.tensor.matmul(
                    cs_ps[:], lhsT=ones_sb[:], rhs=stat_sb[:], start=True, stop=True
                )

                cs_sb = perhead.tile([P, 4], F32, tag="cs_sb")
                nc.vector.tensor_copy(out=cs_sb[:], in_=cs_ps[:])
                scal = perhead.tile([P, 8], F32, tag="scal")
                # scal: 0=mu, 1=Esq, 2=mu^2, 3=var, 4=sd, 5=r, 6=negr
                nc.vector.tensor_tensor(
                    out=scal[:, 0:1], in0=cs_sb[:, 0:1], in1=cs_sb[:, 1:2],
                    op=mybir.AluOpType.add,
                )
                nc.scalar.mul(scal[:, 0:1], scal[:, 0:1], inv_cc)
                nc.vector.tensor_tensor(
                    out=scal[:, 1:2], in0=cs_sb[:, 2:3], in1=cs_sb[:, 3:4],
                    op=mybir.AluOpType.add,
                )
                nc.scalar.mul(scal[:, 1:2], scal[:, 1:2], inv_cc)
                nc.vector.tensor_mul(out=scal[:, 2:3], in0=scal[:, 0:1], in1=scal[:, 0:1])
                nc.vector.tensor_tensor(
                    out=scal[:, 3:4], in0=scal[:, 1:2], in1=scal[:, 2:3],
                    op=mybir.AluOpType.subtract,
                )
                nc.scalar.activation(
                    out=scal[:, 4:5], in_=scal[:, 3:4],
                    func=mybir.ActivationFunctionType.Sqrt,
                    bias=eps_sb[:],
                )
                nc.vector.reciprocal(out=scal[:, 5:6], in_=scal[:, 4:5])
                nc.scalar.mul(scal[:, 6:7], scal[:, 5:6], -1.0)

                # bias_i = -r * rowmax_i ; probs = exp(r*s + bias) (row-sum fused)
                nb_sb = perhead.tile([P, TC], F32, tag="nb")
                se_sb = perhead.tile([P, 2 * TC], F32, tag="se")
                probs_sb = perhead.tile([P, TC, C], F32, tag="probs")
                for mi in range(TC):
                    nc.vector.tensor_mul(
                        out=nb_sb[:, mi : mi + 1],
                        in0=rmax_sb[:, mi : mi + 1],
                        in1=scal[:, 6:7],
                    )
                    nc.scalar.activation(
                        out=probs_sb[:, mi, :],
                        in_=sc_ps[mi][:],
                        func=mybir.ActivationFunctionType.Exp,
                        bias=nb_sb[:, mi : mi + 1],
                        scale=scal[:, 5:6],
                        accum_out=se_sb[:, mi : mi + 1],
                    )
                nc.vector.reciprocal(
                    out=se_sb[:, TC : 2 * TC], in_=se_sb[:, 0:TC]
                )
                for mi in range(TC):
                    nc.vector.tensor_scalar_mul(
                        probs_sb[:, mi, :],
                        probs_sb[:, mi, :],
                        se_sb[:, TC + mi : TC + mi + 1],
                    )
                    # accumulate the head-mean attention map on gpsimd (idle)
                    nc.gpsimd.tensor_add(
                        out=wacc_sb[:, mi, :],
                        in0=wacc_sb[:, mi, :],
                        in1=probs_sb[:, mi, :],
                    )

                # probsT via PE transpose (fp32), then S' += probsT.T-chain @ Wv_h
                probsT_sb = perhead.tile([P, TC, C], DT_SMALL, tag="probsT")
                for ti in range(TC):
                    for tj in range(TC):
                        t_ps = ps.tile([P, P], F32, tag="ps")
                        nc.tensor.transpose(
                            t_ps[:],
                            probs_sb[:, ti, tj * P : (tj + 1) * P],
                            ident_sb[:],
                        )
                        nc.vector.tensor_copy(
                            out=probsT_sb[:, tj, ti * P : (ti + 1) * P], in_=t_ps[:]
                        )

                for mi in range(TC):
                    for kj in range(TC):
                        nc.tensor.matmul(
                            s_acc[mi][:],
                            lhsT=_mm(probsT_sb[:, kj, mi * P : (mi + 1) * P], MM_SMALL),
                            rhs=_mm(wv_sb[:, h, kj, :], MM_SMALL),
                            start=(h == 0 and kj == 0),
                            stop=(h == H - 1 and kj == TC - 1),
                        )

            # ---- epilogue ---------------------------------------------------
            for mi in range(TC):
                nc.scalar.mul(S_sb[:, mi, :], s_acc[mi][:], 1.0 / H)
                nc.scalar.mul(wacc_sb[:, mi, :], wacc_sb[:, mi, :], 1.0 / H)
            nc.sync.dma_start(
                out=wts_h[:].rearrange("(t p) j -> p t j", p=P), in_=wacc_sb[:]
            )

            # Z[c,d] = sum_i S[i,c] * Wout[d,i]
            for mc in range(TC):
                z_ps = ps.tile([P, C], F32, tag="ps")
                for ki in range(TC):
                    nc.tensor.matmul(
                        z_ps[:],
                        lhsT=_mm(S_sb[:, ki, mc * P : (mc + 1) * P], MM_SMALL),
                        rhs=_mm(woutT_sb[:, ki, :], MM_SMALL),
                        start=(ki == 0),
                        stop=(ki == TC - 1),
                    )
                nc.scalar.mul(Z_sb[:, mc, :], z_ps[:], 1.0)

            # O1.T[d, n] = sum_c Z[c, d] * embT[c, n]
            for md in range(TC):
                for nch in range(NCH):
                    o_ps = ps.tile([P, 512], F32, tag="ps")
                    for kc in range(TC):
                        nc.tensor.matmul(
                            o_ps[:],
                            lhsT=_mm(Z_sb[:, kc, md * P : (md + 1) * P], MM_BIG),
                            rhs=_mm(embT_sb[:, kc, nch * 512 : (nch + 1) * 512], MM_BIG),
                            start=(kc == 0),
                            stop=(kc == TC - 1),
                        )
                    o_sb = outs.tile([P, 512], F32, tag="o1")
                    nc.vector.tensor_copy(out=o_sb[:], in_=o_ps[:])
                    nc.sync.dma_start(
                        out=o1T_h[:][
                            md * P : (md + 1) * P, nch * 512 : (nch + 1) * 512
                        ],
                        in_=o_sb[:],
                    )

    nc.compile()
    return nc


_NC_CACHE = None


def kernel(emb1, Wq, Wk, Wv, Wout):
    global _NC_CACHE
    emb1 = np.ascontiguousarray(np.asarray(emb1, dtype=np.float32))
    Wq = np.asarray(Wq, dtype=np.float32)
    Wk = np.asarray(Wk, dtype=np.float32)
    Wv = np.asarray(Wv, dtype=np.float32)
    Wout = np.asarray(Wout, dtype=np.float32)

    wqT = np.ascontiguousarray(Wq.transpose(0, 2, 1))
    wkT = np.ascontiguousarray(Wk.transpose(0, 2, 1))
    wv = np.ascontiguousarray(Wv)
    woutT = np.ascontiguousarray(Wout.T)

    if _NC_CACHE is None:
        _NC_CACHE = build_bass()
    nc = _NC_CACHE

    in_maps = []
    for b in range(B):
        in_maps.append(
            {
                "emb": emb1[b],
                "embT": np.ascontiguousarray(emb1[b].T),
                "wqT": wqT,
                "wkT": wkT,
                "wv": wv,
                "woutT": woutT,
            }
        )

    res = run_bass_kernel_spmd(nc, in_maps, core_ids=list(range(B)))

    O1 = np.empty((B, N, C), dtype=np.float32)
    weights = np.empty((B, C, C), dtype=np.float32)
    for b in range(B):
        O1[b] = res.results[b]["o1T"].T
        weights[b] = res.results[b]["wts"]
    return O1, weights


# revision 9
# speedup vs baseline: 1.7519x; 1.3219x over previous
"""Trainium2 Bass kernel for the per-head channel-attention module.

Math (per batch b, all fp32):
  Q = emb @ Wq[h].T, K = emb @ Wk[h].T        [N, C] each
  scores_h = Q.T @ K / sqrt(C)                [C, C]
  probs = softmax(InstanceNorm(scores), -1)
  weights = mean_h probs                      [C, C]   (output 2)
  O1 = (1/H sum_h probs_h @ V_h-chain) @ Wout [N, C]   (output 1)

Key restructure: scores contract over tokens N, so
  scores_h = Wq_h @ G' @ Wk_h.T  with  G' = (emb.T @ emb) / sqrt(C)
and the output path collapses to
  O1 = emb @ Z;  S'[i,c] = sum_h (probs_h @ Wv_h)[i,c];
  Z[c,d] = (1/H) sum_i S'[i,c] * Wout[d,i]
~11.3 GFLOP/batch -> ~1.5 GFLOP/batch.  InstanceNorm's mean subtraction
cancels inside the row softmax, so only r = rsqrt(var+eps) is needed.

Sharding: data-parallel, one batch per NeuronCore (B=8, 8 cores).
Host pre-transposes emb (embT) and pre-swizzles the weight matrices into
one SBUF-layout buffer so every DMA is a few large contiguous runs.
Per-core outputs: o1T = O1[b].T (host transposes back) and wts = weights[b].
"""

import os

import numpy as np

import concourse.bacc as bacc
import concourse.bass as bass
import concourse.mybir as mybir
import concourse.tile as tile
from concourse.bass_utils import run_bass_kernel_spmd
from concourse.masks import make_identity

B, N, C, H = 8, 4096, 256, 4
EPS = 1e-5
P = 128
TC = C // P          # 2 c-tiles
KT = N // P          # 32 token-tiles
NCH = N // 512       # 8 chunks of 512 tokens for the final matmul
EMB_CHUNKS = 4
F32 = mybir.dt.float32

# Matmul dtype knobs. float32 = exact (4 cyc/row); float32r = fast (~2 cyc/row
# measured) with relaxed multiply precision (~2e-4 rel err end to end).
_DT_MAP = {"float32": mybir.dt.float32, "float32r": mybir.dt.float32r}
MM_BIG = _DT_MAP[os.environ.get("ATT_MM_BIG", "float32r")]     # Gram + O1
MM_SMALL = _DT_MAP[os.environ.get("ATT_MM_SMALL", "float32r")]  # 256^3 matmuls

# weight buffer layout (per-partition f32 element offsets)
WQ_OFF = 0
WK_OFF = WQ_OFF + H * TC * C     # 2048
WV_OFF = WK_OFF + H * TC * C     # 4096
WO_OFF = WV_OFF + H * TC * C     # 6144
WBUF_W = WO_OFF + TC * C         # 6656


def host_pack_weights(Wq, Wk, Wv, Wout):
    """Pack all weights into the exact [128, WBUF_W] SBUF image."""
    def swz(a):  # [X, 2, 128, Y] -> [128, X*2*Y]
        return np.ascontiguousarray(
            a.reshape(-1, TC, P, C).transpose(2, 0, 1, 3).reshape(P, -1)
        )

    wq = swz(Wq.transpose(0, 2, 1))   # [p, h*tc*d] = Wq[h, d, tc*128+p]
    wk = swz(Wk.transpose(0, 2, 1))
    wv = swz(Wv)                      # [p, h*tc*c] = Wv[h, tc*128+p, c]
    wo = swz(Wout.T[None])            # [p, tc*d] = Wout[d, tc*128+p]
    return np.ascontiguousarray(np.concatenate([wq, wk, wv, wo], axis=1))


def build_bass():
    nc = bacc.Bacc(None, target_bir_lowering=False)

    emb_h = nc.dram_tensor("emb", [N, C], MM_BIG, kind="ExternalInput")
    embT_h = nc.dram_tensor("embT", [C, N], MM_BIG, kind="ExternalInput")
    wbuf_h = nc.dram_tensor("wbuf", [P, WBUF_W], MM_SMALL, kind="ExternalInput")
    o1T_h = nc.dram_tensor("o1T", [C, N], F32, kind="ExternalOutput")
    wts_h = nc.dram_tensor("wts", [C, C], F32, kind="ExternalOutput")

    with tile.TileContext(nc) as tc:
        with (
            tc.tile_pool(name="singles", bufs=1) as singles,
            tc.tile_pool(name="perhead", bufs=2) as perhead,
            tc.tile_pool(name="outs", bufs=3) as outs,
            tc.tile_pool(name="psc", bufs=4, space="PSUM") as psc,
            tc.tile_pool(name="ps", bufs=2, space="PSUM") as ps,
            tc.tile_pool(name="acc", bufs=2, space="PSUM") as acc,
        ):
            # ---- resident SBUF tensors -------------------------------------
            emb_sb = singles.tile([P, KT, C], MM_BIG)    # emb[p*32+t, c]
            embT_sb = singles.tile([P, TC, N], MM_BIG)   # emb[n, t*128+p]
            wbuf_sb = singles.tile([P, WBUF_W], MM_SMALL)
            G_sb = singles.tile([P, TC, C], MM_SMALL)    # G/sqrt(C), [c', (tc,c)]
            S_sb = singles.tile([P, TC, C], MM_SMALL)    # S'/H
            Z_sb = singles.tile([P, TC, C], MM_BIG)
            probs_sb = singles.tile([P, 2 * H, C], F32)  # [i, (2h+mi), j]
            wacc_sb = singles.tile([P, TC, C], F32)
            stat_sb = singles.tile([P, 2 * H], F32)      # sums | sumsqs
            rmax_sb = singles.tile([P, H, TC], F32)
            nb_sb = singles.tile([P, H, TC], F32)
            se_sb = singles.tile([P, 2 * H], F32)        # exp row sums
            rse_sb = singles.tile([P, 2 * H], F32)
            scal_sb = singles.tile([P, 4 * H], F32)      # mu|Esq|var~|r
            ones_sb = singles.tile([P, P], F32)
            ident_sb = singles.tile([P, P], F32)
            eps_sb = singles.tile([P, 1], F32)

            nc.vector.memset(ones_sb[:], 1.0)
            nc.vector.memset(eps_sb[:], EPS)
            make_identity(nc, ident_sb[:])

            def wq_ap(h, t):
                return wbuf_sb[:, WQ_OFF + (h * TC + t) * C : WQ_OFF + (h * TC + t + 1) * C]

            def wk_ap(h, t):
                return wbuf_sb[:, WK_OFF + (h * TC + t) * C : WK_OFF + (h * TC + t + 1) * C]

            def wv_ap(h, t):
                return wbuf_sb[:, WV_OFF + (h * TC + t) * C : WV_OFF + (h * TC + t + 1) * C]

            def wo_ap(t):
                return wbuf_sb[:, WO_OFF + t * C : WO_OFF + (t + 1) * C]

            # ---- input DMAs (emb chunked so Gram starts early) -------------
            emb_dram = emb_h[:].rearrange("(p t) c -> p t c", p=P)
            TPC = KT // EMB_CHUNKS
            for ch in range(EMB_CHUNKS):
                nc.sync.dma_start(
                    out=emb_sb[:, ch * TPC : (ch + 1) * TPC, :],
                    in_=emb_dram[:, ch * TPC : (ch + 1) * TPC, :],
                )
            nc.sync.dma_start(out=wbuf_sb[:], in_=wbuf_h[:])
            nc.sync.dma_start(
                out=embT_sb[:], in_=embT_h[:].rearrange("(t p) n -> p t n", p=P)
            )

            # ---- Gram: G = emb.T @ emb, scaled by 1/sqrt(C) ----------------
            # token-partition per tile t is {p*32+t}; any partition of the
            # 4096 tokens is valid for the Gram contraction
            g_ps = [ps.tile([P, C], F32, tag="ps", name=f"g{i}") for i in range(TC)]
            for k in range(KT):
                for mi in range(TC):
                    nc.tensor.matmul(
                        g_ps[mi][:],
                        lhsT=emb_sb[:, k, mi * P : (mi + 1) * P],
                        rhs=emb_sb[:, k, :],
                        start=(k == 0),
                        stop=(k == KT - 1),
                    )
            for mi in range(TC):
                nc.vector.tensor_scalar_mul(G_sb[:, mi, :], g_ps[mi][:], 1.0 / 16.0)

            # S' accumulator lives across the whole head loop
            s_acc = [
                acc.tile([P, C], F32, tag="acc", name=f"sacc{i}") for i in range(TC)
            ]

            inv_cc = 1.0 / float(C * C)

            # ---- phase 1: all heads' U and scores (PE stays dense) ---------
            sc_ps = []
            for h in range(H):
                U_sb = perhead.tile([P, TC, C], MM_SMALL, tag="u")
                for mc in range(TC):
                    u_ps = ps.tile([P, C], F32, tag="ps")
                    for kc in range(TC):
                        nc.tensor.matmul(
                            u_ps[:],
                            lhsT=G_sb[:, kc, mc * P : (mc + 1) * P],
                            rhs=wk_ap(h, kc),
                            start=(kc == 0),
                            stop=(kc == TC - 1),
                        )
                    nc.vector.tensor_copy(out=U_sb[:, mc, :], in_=u_ps[:])

                p_ = psc.tile([P, TC, C], F32, tag="sc")
                for mi in range(TC):
                    for kc in range(TC):
                        nc.tensor.matmul(
                            p_[:, mi, :],
                            lhsT=wq_ap(h, kc)[:, mi * P : (mi + 1) * P],
                            rhs=U_sb[:, kc, :],
                            start=(kc == 0),
                            stop=(kc == TC - 1),
                        )
                sc_ps.append(p_)

                # stats stream in behind the score matmuls
                sq_scratch = perhead.tile([P, TC, C], F32, tag="sqs")
                nc.vector.reduce_sum(
                    out=stat_sb[:, h : h + 1],
                    in_=p_[:],
                    axis=mybir.AxisListType.XY,
                )
                nc.scalar.activation(
                    out=sq_scratch[:],
                    in_=p_[:],
                    func=mybir.ActivationFunctionType.Square,
                    accum_out=stat_sb[:, H + h : H + h + 1],
                )
                nc.vector.reduce_max(
                    out=rmax_sb[:, h, :],
                    in_=p_[:],
                    axis=mybir.AxisListType.X,
                )

            # ---- one cross-partition reduction + one batched norm chain ----
            cs_ps = ps.tile([P, 2 * H], F32, tag="ps")
            nc.tensor.matmul(
                cs_ps[:], lhsT=ones_sb[:], rhs=stat_sb[:], start=True, stop=True
            )
            mu = scal_sb[:, 0:H]
            esq = scal_sb[:, H : 2 * H]
            var = scal_sb[:, 2 * H : 3 * H]
            rr = scal_sb[:, 3 * H : 4 * H]
            nc.vector.tensor_scalar_mul(mu, cs_ps[:, 0:H], inv_cc)
            nc.vector.tensor_scalar_mul(esq, cs_ps[:, H : 2 * H], inv_cc)
            nc.vector.tensor_mul(out=var, in0=mu, in1=mu)
            nc.vector.tensor_tensor(
                out=var, in0=esq, in1=var, op=mybir.AluOpType.subtract
            )
            nc.scalar.activation(
                out=var, in_=var,
                func=mybir.ActivationFunctionType.Sqrt,
                bias=eps_sb[:],
            )
            nc.vector.reciprocal(out=rr, in_=var)
            # nb[h, mi] = -r_h * rowmax[h, mi]
            nc.vector.tensor_tensor(
                out=nb_sb[:],
                in0=rmax_sb[:],
                in1=rr[:, :, None].to_broadcast([P, H, TC]),
                op=mybir.AluOpType.mult,
            )
            nc.vector.tensor_scalar_mul(nb_sb[:], nb_sb[:], -1.0)

            # ---- phase 2 per head: exp/normalize -> transpose -> S' --------
            for h in range(H):
                for mi in range(TC):
                    nc.scalar.activation(
                        out=probs_sb[:, TC * h + mi, :],
                        in_=sc_ps[h][:, mi, :],
                        func=mybir.ActivationFunctionType.Exp,
                        bias=nb_sb[:, h, mi : mi + 1],
                        scale=rr[:, h : h + 1],
                        accum_out=se_sb[:, TC * h + mi : TC * h + mi + 1],
                    )
                nc.vector.reciprocal(
                    out=rse_sb[:, TC * h : TC * h + TC],
                    in_=se_sb[:, TC * h : TC * h + TC],
                )
                nc.vector.tensor_tensor(
                    out=probs_sb[:, TC * h : TC * h + TC, :],
                    in0=probs_sb[:, TC * h : TC * h + TC, :],
                    in1=rse_sb[:, TC * h : TC * h + TC, None].to_broadcast(
                        [P, TC, C]
                    ),
                    op=mybir.AluOpType.mult,
                )

                probsT_sb = perhead.tile([P, TC, C], MM_SMALL, tag="probsT")
                for ti in range(TC):
                    for tj in range(TC):
                        t_ps = ps.tile([P, P], F32, tag="ps")
                        nc.tensor.transpose(
                            t_ps[:],
                            probs_sb[:, TC * h + ti, tj * P : (tj + 1) * P],
                            ident_sb[:],
                        )
                        nc.vector.tensor_copy(
                            out=probsT_sb[:, tj, ti * P : (ti + 1) * P], in_=t_ps[:]
                        )

                for mi in range(TC):
                    for kj in range(TC):
                        nc.tensor.matmul(
                            s_acc[mi][:],
                            lhsT=probsT_sb[:, kj, mi * P : (mi + 1) * P],
                            rhs=wv_ap(h, kj),
                            start=(h == 0 and kj == 0),
                            stop=(h == H - 1 and kj == TC - 1),
                        )

            # weights output: mean over heads via a strided free-dim reduce
            nc.vector.reduce_sum(
                out=wacc_sb[:],
                in_=probs_sb[:].rearrange("p (h m) j -> p m j h", h=H),
                axis=mybir.AxisListType.X,
            )
            nc.gpsimd.tensor_scalar_mul(wacc_sb[:], wacc_sb[:], 1.0 / H)
            nc.sync.dma_start(
                out=wts_h[:].rearrange("(t p) j -> p t j", p=P), in_=wacc_sb[:]
            )

            # ---- epilogue: Z then O1 ---------------------------------------
            for mi in range(TC):
                nc.vector.tensor_scalar_mul(S_sb[:, mi, :], s_acc[mi][:], 1.0 / H)
            for mc in range(TC):
                z_ps = ps.tile([P, C], F32, tag="ps")
                for ki in range(TC):
                    nc.tensor.matmul(
                        z_ps[:],
                        lhsT=S_sb[:, ki, mc * P : (mc + 1) * P],
                        rhs=wo_ap(ki),
                        start=(ki == 0),
                        stop=(ki == TC - 1),
                    )
                nc.vector.tensor_copy(out=Z_sb[:, mc, :], in_=z_ps[:])

            # O1.T[d, n] = sum_c Z[c, d] * embT[c, n]
            for md in range(TC):
                for nch in range(NCH):
                    o_ps = ps.tile([P, 512], F32, tag="ps")
                    for kc in range(TC):
                        nc.tensor.matmul(
                            o_ps[:],
                            lhsT=Z_sb[:, kc, md * P : (md + 1) * P],
                            rhs=embT_sb[:, kc, nch * 512 : (nch + 1) * 512],
                            start=(kc == 0),
                            stop=(kc == TC - 1),
                        )
                    o_sb = outs.tile([P, 512], F32, tag="o1")
                    nc.vector.tensor_copy(out=o_sb[:], in_=o_ps[:])
                    nc.sync.dma_start(
                        out=o1T_h[:][
                            md * P : (md + 1) * P, nch * 512 : (nch + 1) * 512
                        ],
                        in_=o_sb[:],
                    )

    nc.compile()
    return nc


_NC_CACHE = None


def host_in_maps(emb1, Wq, Wk, Wv, Wout):
    wbuf = host_pack_weights(Wq, Wk, Wv, Wout)
    in_maps = []
    for b in range(B):
        in_maps.append(
            {
                "emb": np.ascontiguousarray(emb1[b]),
                "embT": np.ascontiguousarray(emb1[b].T),
                "wbuf": wbuf,
            }
        )
    return in_maps


def kernel(emb1, Wq, Wk, Wv, Wout):
    global _NC_CACHE
    emb1 = np.ascontiguousarray(np.asarray(emb1, dtype=np.float32))
    Wq = np.asarray(Wq, dtype=np.float32)
    Wk = np.asarray(Wk, dtype=np.float32)
    Wv = np.asarray(Wv, dtype=np.float32)
    Wout = np.asarray(Wout, dtype=np.float32)

    if _NC_CACHE is None:
        _NC_CACHE = build_bass()
    nc = _NC_CACHE

    in_maps = host_in_maps(emb1, Wq, Wk, Wv, Wout)
    res = run_bass_kernel_spmd(nc, in_maps, core_ids=list(range(B)))

    O1 = np.empty((B, N, C), dtype=np.float32)
    weights = np.empty((B, C, C), dtype=np.float32)
    for b in range(B):
        O1[b] = res.results[b]["o1T"].T
        weights[b] = res.results[b]["wts"]
    return O1, weights
